# revision 5
# baseline (speedup 1.0000x reference)
"""Two-layer GraphSAGE (mean aggr, BatchNorm folded, eval) on 8 trn2 cores.

Strategy (per 25000-node shard, edges partitioned by destination):
  z = h @ Wl_f (fp16 table, node-major, per-shard)  -> AllGather -> full table
  agg[dst] = sum_e z[src_e] computed as: per-source-shard dma_gather (int16
    local indices) pulls edge rows into SBUF; PE matmuls with on-chip-built
    one-hot selection matrices (DVE iota==dstcmp) scatter-accumulate them
    into 512-destination-wide fp32 PSUM windows; windows land in a resident
    SBUF accumulator.
  h_next = relu(agg * invdeg + rT + c)  (DVE tensor ops + ACT relu w/ bias)
Everything flows feature-major (128 features on partitions) so layer-2
matmuls need no transposes; the host transposes the final [128, N] output.
BN is folded into the weights on the host; all per-edge index structures are
host-precomputed and baked into the NEFF / shipped as int16/f32 inputs.
"""
import sys
sys.path.insert(0, '/opt/trn_rl_repo')

import numpy as np

N = 200000
E = 600000
C = 8
P = N // C              # 25000 nodes per core
NB = 196
SL = NB * 128           # 25088 padded slots
WIN = 512
NWIN = SL // WIN        # 49
GW = 4                  # windows per PSUM group
NG = (NWIN + GW - 1) // GW
IN_DIM = 130
HID = 128
BN_EPS = 1e-5
MAXNI = 1024            # dma_gather descriptor-ring limit

F16 = np.float16

_cache = {}


# ---------------------------------------------------------------------------
# host-side graph preprocessing (pure integer/index work)
# ---------------------------------------------------------------------------

def _preprocess_graph(edge_index):
    src = edge_index[0].astype(np.int64)
    dst = edge_index[1].astype(np.int64)
    deg = np.bincount(dst, minlength=N)

    owner = dst // P
    dloc = dst - owner * P
    jsh = src // P
    sloc = src - jsh * P
    # table row of a node within its shard block (block-transposed layout
    # so phase-A staging DMAs write contiguous per-partition runs)
    srow = (sloc % 128) * NB + sloc // 128
    win = dloc // WIN
    grp = win // GW

    # per (core, group, shard) edge counts
    counts = np.zeros((C, NG, C), np.int64)
    np.add.at(counts, (owner, grp, jsh), 1)
    maxc = counts.max(axis=0)                      # [NG, C]
    S = ((maxc + 127) // 128) * 128                # padded call sizes

    # order edges by (owner, group, shard, dst_local)
    eorder = np.lexsort((dloc, jsh, grp, owner))
    o_s, g_s, j_s, dl_s, w_s, sr_s = (owner[eorder], grp[eorder], jsh[eorder],
                                      dloc[eorder], win[eorder], srow[eorder])

    ZR = 25087  # per-shard zero row (block-transposed row of pad node 25087)

    calls = []          # static: list of dicts per (g, j) call
    TOTC = 0
    for g in range(NG):
        for j in range(C):
            s = int(S[g, j])
            if s == 0:
                continue
            calls.append(dict(g=g, j=j, S=s, colofs=TOTC // 16))
            TOTC += s

    gidx = np.full((C, TOTC), ZR, np.int32)        # linear per-call idx lists
    dstw = np.full((C, TOTC), -1, np.int64)        # global window per edge
    dcmp = np.full((C, TOTC), -1.0, np.float32)    # dst_local - win*WIN

    # fill per-core slices
    start = np.searchsorted(o_s * (NG * C) + g_s * C + j_s,
                            np.arange(C * NG * C))
    key_s = o_s * (NG * C) + g_s * C + j_s
    bounds = np.searchsorted(key_s, np.arange(C * NG * C + 1))
    for ci in range(C):
        ofs = 0
        for call in calls:
            g, j, s = call['g'], call['j'], call['S']
            k = ci * (NG * C) + g * C + j
            a, b = bounds[k], bounds[k + 1]
            n = b - a
            gidx[ci, ofs:ofs + n] = sr_s[a:b]
            dstw[ci, ofs:ofs + n] = w_s[a:b]
            dcmp[ci, ofs:ofs + n] = dl_s[a:b] - w_s[a:b] * WIN
            ofs += s
    assert ofs == TOTC

    # pieces: per call, per 128-edge tile, the set of windows touched by ANY
    # core -> (call_idx, tile, window, piece_col); dstcmp input col per piece
    pieces = []
    NP = 0
    for cidx, call in enumerate(calls):
        g, s = call['g'], call['S']
        base = call['colofs'] * 16
        call['tiles'] = []
        for t in range(s // 128):
            lo, hi = base + t * 128, base + (t + 1) * 128
            ws = np.unique(dstw[:, lo:hi])
            ws = ws[ws >= 0]
            tp = []
            for w in ws:
                tp.append((int(w), NP))
                NP += 1
            call['tiles'].append(tp)
            pieces.append((cidx, t, tp))

    # per-edge 1/max(deg,1) in the same linear layout
    einv = np.zeros((C, TOTC), np.float32)
    dmax_all = 1.0 / np.maximum(deg, 1).astype(np.float32)
    for ci in range(C):
        ofs = 0
        for call in calls:
            g, j, s = call['g'], call['j'], call['S']
            k = ci * (NG * C) + g * C + j
            a, b = bounds[k], bounds[k + 1]
            einv[ci, ofs:ofs + (b - a)] = dmax_all[ci * P + dl_s[a:b]]
            ofs += s

    # dstcmp/dinv inputs [128, NP]: for piece (call,t,w): col p = edge t*128+p
    dcmp_in = np.full((C, 128, NP), -1.0, np.float32)
    dinv_in = np.zeros((C, 128, NP), np.float32)
    for cidx, call in enumerate(calls):
        base = call['colofs'] * 16
        for t, tp in enumerate(call['tiles']):
            lo = base + t * 128
            seg_w = dstw[:, lo:lo + 128]           # [C, 128]
            seg_c = dcmp[:, lo:lo + 128]
            seg_i = einv[:, lo:lo + 128]
            for (w, pc) in tp:
                m = seg_w == w
                dcmp_in[:, :, pc] = np.where(m, seg_c, -1.0)
                dinv_in[:, :, pc] = np.where(m, seg_i, 0.0)

    # wrap gidx into dma_gather layout: [128, TOTC//16] int16, per call block
    gidx_in = np.zeros((C, 128, TOTC // 16), np.int16)
    for call in calls:
        s = call['S']
        base = call['colofs'] * 16
        cb = call['colofs']
        seg = gidx[:, base:base + s].astype(np.int16)          # [C, s]
        blk = seg.reshape(C, s // 16, 16).transpose(0, 2, 1)   # [C, 16, s/16]
        gidx_in[:, :, cb:cb + s // 16] = np.tile(blk, (1, 8, 1))

    return dict(calls=calls, TOTC=TOTC, NP=NP, gidx_in=gidx_in,
                dcmp_in=dcmp_in, dinv_in=dinv_in)


def _fold_weights(Wl, bl, Wr, g, be, rm, rv):
    s = (np.asarray(g) / np.sqrt(np.asarray(rv) + BN_EPS)).astype(np.float32)
    Wl_f = (np.asarray(Wl) * s[None, :]).astype(np.float32)
    Wr_f = (np.asarray(Wr) * s[None, :]).astype(np.float32)
    c = ((np.asarray(bl) - np.asarray(rm)) * s + np.asarray(be)).astype(np.float32)
    return Wl_f, Wr_f, c


# ---------------------------------------------------------------------------
# bass kernel
# ---------------------------------------------------------------------------

def _build_kernel(calls, TOTC, NP, nreps=1):
    import concourse.bacc as bacc
    import concourse.tile as tile
    import concourse.mybir as mybir

    F32 = mybir.dt.float32
    FP16 = mybir.dt.float16
    I16 = mybir.dt.int16
    AF = mybir.ActivationFunctionType
    ALU = mybir.AluOpType

    nc = bacc.Bacc("TRN2", target_bir_lowering=False, debug=False, num_devices=C)

    xTa = nc.dram_tensor("xTa", [128, SL], FP16, kind="ExternalInput")
    xTb = nc.dram_tensor("xTb", [2, SL], FP16, kind="ExternalInput")
    W1la = nc.dram_tensor("W1la", [128, 128], FP16, kind="ExternalInput")
    W1lb = nc.dram_tensor("W1lb", [2, 128], FP16, kind="ExternalInput")
    W1ra = nc.dram_tensor("W1ra", [128, 128], FP16, kind="ExternalInput")
    W1rb = nc.dram_tensor("W1rb", [2, 128], FP16, kind="ExternalInput")
    W2l = nc.dram_tensor("W2l", [128, 128], FP16, kind="ExternalInput")
    W2r = nc.dram_tensor("W2r", [128, 128], FP16, kind="ExternalInput")
    c1i = nc.dram_tensor("c1", [128, 1], F32, kind="ExternalInput")
    c2i = nc.dram_tensor("c2", [128, 1], F32, kind="ExternalInput")
    gidxi = nc.dram_tensor("gidx", [128, TOTC // 16], I16, kind="ExternalInput")
    dcmpi = nc.dram_tensor("dcmp", [128, NP], F32, kind="ExternalInput")
    dinvi = nc.dram_tensor("dinv", [128, NP], F32, kind="ExternalInput")
    iotai = nc.dram_tensor("iota", [128, WIN], F32, kind="ExternalInput")
    hout = nc.dram_tensor("houtT", [128, SL], F32, kind="ExternalOutput")

    inb = nc.dram_tensor("inb", [SL, 128], FP16)
    rdr1 = nc.dram_tensor("rdr1", [128, SL], F32)
    rdr2 = nc.dram_tensor("rdr2", [128, SL], F32)
    ztab = nc.dram_tensor("ztab", [C * SL, 128], FP16, addr_space="Shared")

    CH = 4  # node blocks per z-staging chunk (r-chunk = 512 = PSUM width)

    with tile.TileContext(nc) as tc:
        with (
            tc.tile_pool(name="const", bufs=1) as cons,
            tc.tile_pool(name="big", bufs=1) as bigp,
            tc.tile_pool(name="gch", bufs=3) as gchp,
            tc.tile_pool(name="oh", bufs=4) as ohp,
            tc.tile_pool(name="xs", bufs=2) as xsp,
            tc.tile_pool(name="st", bufs=3) as stp,
            tc.tile_pool(name="sm", bufs=3) as smp,
            tc.tile_pool(name="agps", bufs=GW + 2, space="PSUM") as agps,
            tc.tile_pool(name="zrps", bufs=2, space="PSUM") as zrps,
        ):
            w1la = cons.tile([128, 128], FP16)
            w1lb = cons.tile([2, 128], FP16)
            w1ra = cons.tile([128, 128], FP16)
            w1rb = cons.tile([2, 128], FP16)
            w2l = cons.tile([128, 128], FP16)
            w2r = cons.tile([128, 128], FP16)
            c1t = cons.tile([128, 1], F32)
            c2t = cons.tile([128, 1], F32)
            gidx_t = cons.tile([128, TOTC // 16], I16)
            dcmp_t = cons.tile([128, NP], F32)
            dinv_t = cons.tile([128, NP], F32)
            iota_t = cons.tile([128, WIN], F32)
            for sb, dr in ((w1la, W1la), (w1lb, W1lb), (w1ra, W1ra),
                           (w1rb, W1rb), (w2l, W2l), (w2r, W2r),
                           (c1t, c1i), (c2t, c2i), (gidx_t, gidxi),
                           (dcmp_t, dcmpi), (dinv_t, dinvi), (iota_t, iotai)):
                nc.sync.dma_start(sb[:], dr[:])

            agg = bigp.tile([128, SL], F32)

            def inb_blocks(b0, nb):
                # table rows p*NB + (b0..b0+nb) -> [128, nb, 128]
                return inb[:].rearrange("(p b) f -> p b f", p=128)[:, b0:b0 + nb, :]

            def aggregate(layer):
                nc.gpsimd.collective_compute(
                    "AllGather", mybir.AluOpType.bypass,
                    replica_groups=[list(range(C))],
                    ins=[inb[:].opt()], outs=[ztab[:].opt()])
                cur_g = -1
                open_ps = {}

                def flush(gdone):
                    for w, (ps, _) in sorted(open_ps.items()):
                        nc.vector.tensor_copy(
                            agg[:, w * WIN:(w + 1) * WIN], ps[:])
                    if gdone >= 0:
                        for w in range(gdone * GW, min(NWIN, (gdone + 1) * GW)):
                            if w not in open_ps:
                                nc.vector.memset(
                                    agg[:, w * WIN:(w + 1) * WIN], 0.0)

                for call in calls:
                    g, j, S = call['g'], call['j'], call['S']
                    if g != cur_g:
                        flush(cur_g)
                        open_ps = {}
                        cur_g = g
                    T = S // 128
                    gch = gchp.tile([128, MAXNI // 128, 128], FP16, tag="gch")
                    nc.gpsimd.dma_gather(
                        out_ap=gch[:, :T, :],
                        in_ap=ztab[j * SL:(j + 1) * SL, :],
                        idxs_ap=gidx_t[:, call['colofs']:call['colofs'] + S // 16],
                        num_idxs=S, num_idxs_reg=S, elem_size=128)
                    for t, tp in enumerate(call['tiles']):
                        for (w, pc) in tp:
                            oh = ohp.tile([128, WIN], FP16, tag="oh")
                            nc.vector.tensor_scalar(
                                oh[:], iota_t[:],
                                dcmp_t[:, pc:pc + 1], dinv_t[:, pc:pc + 1],
                                op0=ALU.is_equal, op1=ALU.mult)
                            if w not in open_ps:
                                ps = agps.tile([128, WIN], F32, tag="agps")
                                open_ps[w] = (ps, True)
                                first = True
                            else:
                                ps, _ = open_ps[w]
                                first = False
                            nc.tensor.matmul(ps[:], lhsT=gch[:, t, :], rhs=oh[:],
                                             start=first, stop=False,
                                             skip_group_check=True)
                flush(cur_g)

            for rep in range(nreps):
                # ---- phase A: z1 (node-major, fp16) + r1T (feat-major) ----
                for cw in range(NWIN):
                    b0 = cw * CH
                    xa = xsp.tile([128, WIN], FP16, tag="xa")
                    xb = xsp.tile([2, WIN], FP16, tag="xb")
                    nc.sync.dma_start(xa[:], xTa[:, cw * WIN:(cw + 1) * WIN])
                    nc.sync.dma_start(xb[:], xTb[:, cw * WIN:(cw + 1) * WIN])
                    zt = stp.tile([128, CH, 128], FP16, tag="zt")
                    for k in range(CH):
                        ps = zrps.tile([128, WIN], F32, tag="zrps")
                        nc.tensor.matmul(ps[:, 0:128], lhsT=xa[:, k * 128:(k + 1) * 128],
                                         rhs=w1la[:], start=True, stop=False)
                        nc.tensor.matmul(ps[:, 0:128], lhsT=xb[:, k * 128:(k + 1) * 128],
                                         rhs=w1lb[:], start=False, stop=True)
                        nc.scalar.copy(zt[:, k, :], ps[:, 0:128])
                    nc.sync.dma_start(inb_blocks(b0, CH), zt[:])
                    # r1T chunk for the same nodes
                    ps2 = zrps.tile([128, WIN], F32, tag="zrps")
                    nc.tensor.matmul(ps2[:], lhsT=w1ra[:],
                                     rhs=xa[:], start=True, stop=False)
                    nc.tensor.matmul(ps2[:], lhsT=w1rb[:],
                                     rhs=xb[:], start=False, stop=True)
                    rt = stp.tile([128, WIN], F32, tag="rt")
                    nc.vector.tensor_copy(rt[:], ps2[:])
                    nc.sync.dma_start(rdr1[:, cw * WIN:(cw + 1) * WIN], rt[:])

                # ---- layer 1 aggregation ----
                aggregate(1)

                # ---- phase D: finalize h1T + layer-2 z2/r2T ----
                for cw in range(NWIN):
                    rc = smp.tile([128, WIN], F32, tag="rc")
                    nc.sync.dma_start(rc[:], rdr1[:, cw * WIN:(cw + 1) * WIN])
                    u = smp.tile([128, WIN], F32, tag="u")
                    nc.vector.tensor_add(u[:], agg[:, cw * WIN:(cw + 1) * WIN], rc[:])
                    h1 = smp.tile([128, WIN], FP16, tag="h1")
                    nc.scalar.activation(h1[:], u[:], AF.Relu, bias=c1t[:, 0:1])
                    if (cw + 1) * WIN > P:
                        nc.vector.memset(h1[:, P - cw * WIN:], 0.0)
                    # z2 for these 4 node blocks
                    zt = stp.tile([128, 4, 128], FP16, tag="z2t")
                    for k in range(4):
                        ps = zrps.tile([128, WIN], F32, tag="zrps")
                        nc.tensor.matmul(ps[:, 0:128], lhsT=h1[:, k * 128:(k + 1) * 128],
                                         rhs=w2l[:], start=True, stop=True)
                        nc.scalar.copy(zt[:, k, :], ps[:, 0:128])
                    nc.sync.dma_start(inb_blocks(cw * 4, 4), zt[:])
                    ps2 = zrps.tile([128, WIN], F32, tag="zrps")
                    nc.tensor.matmul(ps2[:], lhsT=w2r[:], rhs=h1[:],
                                     start=True, stop=True)
                    r2 = smp.tile([128, WIN], F32, tag="rc")
                    nc.vector.tensor_copy(r2[:], ps2[:])
                    nc.sync.dma_start(rdr2[:, cw * WIN:(cw + 1) * WIN], r2[:])

                # ---- layer 2 aggregation ----
                aggregate(2)

                # ---- phase G: finalize h2T -> output ----
                for cw in range(NWIN):
                    rc = smp.tile([128, WIN], F32, tag="rc")
                    nc.sync.dma_start(rc[:], rdr2[:, cw * WIN:(cw + 1) * WIN])
                    u = smp.tile([128, WIN], F32, tag="u")
                    nc.vector.tensor_add(u[:], agg[:, cw * WIN:(cw + 1) * WIN], rc[:])
                    h2 = smp.tile([128, WIN], F32, tag="h2o")
                    nc.scalar.activation(h2[:], u[:], AF.Relu, bias=c2t[:, 0:1])
                    nc.sync.dma_start(hout[:, cw * WIN:(cw + 1) * WIN], h2[:])
    nc.compile()
    return nc


# ---------------------------------------------------------------------------
# PJRT runner (cached jit, sharded over the 8 cores)
# ---------------------------------------------------------------------------

class _SpmdRunner:
    def __init__(self, nc, n_cores=C):
        import jax
        from jax.sharding import Mesh, PartitionSpec
        from jax.experimental.shard_map import shard_map
        import concourse.mybir as mybir
        from concourse.bass2jax import (_bass_exec_p, install_neuronx_cc_hook,
                                        partition_id_tensor)
        self.jax = jax
        install_neuronx_cc_hook()
        self.n_cores = n_cores
        partition_name = nc.partition_id_tensor.name if nc.partition_id_tensor else None
        in_names, out_names, out_avals, zero_outs = [], [], [], []
        for alloc in nc.m.functions[0].allocations:
            if not isinstance(alloc, mybir.MemoryLocationSet):
                continue
            name = alloc.memorylocations[0].name
            if alloc.kind == "ExternalInput":
                if name != partition_name:
                    in_names.append(name)
            elif alloc.kind == "ExternalOutput":
                shape = tuple(alloc.tensor_shape)
                dtype = mybir.dt.np(alloc.dtype)
                out_names.append(name)
                out_avals.append(jax.core.ShapedArray(shape, dtype))
                zero_outs.append(np.zeros(shape, dtype))
        self.in_names, self.out_names = in_names, out_names
        self.out_avals, self.zero_outs = out_avals, zero_outs
        n_params = len(in_names)
        all_in_names = list(in_names) + list(out_names)
        if partition_name is not None:
            all_in_names.append(partition_name)

        def _body(*args):
            operands = list(args)
            if partition_name is not None:
                operands.append(partition_id_tensor())
            outs = _bass_exec_p.bind(
                *operands,
                out_avals=tuple(out_avals),
                in_names=tuple(all_in_names),
                out_names=tuple(out_names),
                lowering_input_output_aliases=(),
                sim_require_finite=False,
                sim_require_nnan=False,
                nc=nc,
            )
            return tuple(outs)

        devices = jax.devices()[:n_cores]
        self.mesh = Mesh(np.asarray(devices), ("core",))
        in_specs = (PartitionSpec("core"),) * (n_params + len(out_names))
        out_specs = (PartitionSpec("core"),) * len(out_names)
        self.fn = jax.jit(
            shard_map(_body, mesh=self.mesh, in_specs=in_specs,
                      out_specs=out_specs, check_rep=False),
            keep_unused=True,
        )
        self.PartitionSpec = PartitionSpec

    def upload(self, in_maps):
        jax = self.jax
        n = self.n_cores
        per_core = [[np.asarray(m[name]) for name in self.in_names] for m in in_maps]
        concat_in = [np.concatenate([per_core[c][i] for c in range(n)], axis=0)
                     for i in range(len(self.in_names))]
        concat_zeros = [np.zeros((n * z.shape[0], *z.shape[1:]), z.dtype)
                        for z in self.zero_outs]
        sharding = jax.sharding.NamedSharding(self.mesh, self.PartitionSpec("core"))
        self.args = [jax.device_put(a, sharding) for a in concat_in + concat_zeros]

    def call_outputs(self):
        jax = self.jax
        outs = self.fn(*self.args)
        jax.block_until_ready(outs)
        n = self.n_cores
        return [
            {name: np.asarray(outs[i]).reshape(n, *self.out_avals[i].shape)[c]
             for i, name in enumerate(self.out_names)}
            for c in range(n)
        ]


# ---------------------------------------------------------------------------
# public entry point
# ---------------------------------------------------------------------------

def _prepare(edge_index, nreps=1):
    key = (hash(edge_index.tobytes()), nreps)
    if key in _cache:
        return _cache[key]
    g = _preprocess_graph(edge_index)
    nc = _build_kernel(g['calls'], g['TOTC'], g['NP'], nreps=nreps)
    runner = _SpmdRunner(nc)
    _cache[key] = (g, runner)
    return g, runner


def _make_in_maps(inputs, g):
    x = np.asarray(inputs['x'], np.float32)
    W1l_f, W1r_f, c1 = _fold_weights(
        inputs['W1_l'], inputs['b1_l'], inputs['W1_r'], inputs['g1'],
        inputs['be1'], inputs['rm1'], inputs['rv1'])
    W2l_f, W2r_f, c2 = _fold_weights(
        inputs['W2_l'], inputs['b2_l'], inputs['W2_r'], inputs['g2'],
        inputs['be2'], inputs['rm2'], inputs['rv2'])

    shared = {
        'W1la': W1l_f[0:128].astype(F16),
        'W1lb': W1l_f[128:130].astype(F16),
        'W1ra': W1r_f[0:128].astype(F16),
        'W1rb': W1r_f[128:130].astype(F16),
        'W2l': W2l_f.astype(F16),
        'W2r': W2r_f.astype(F16),
        'c1': c1.reshape(128, 1),
        'c2': c2.reshape(128, 1),
        'iota': np.tile(np.arange(WIN, dtype=np.float32), (128, 1)),
    }
    in_maps = []
    for i in range(C):
        xT = np.zeros((IN_DIM, SL), np.float32)
        xT[:, :P] = x[i * P:(i + 1) * P].T
        m = dict(shared)
        m['xTa'] = xT[0:128].astype(F16)
        m['xTb'] = xT[128:130].astype(F16)
        m['gidx'] = g['gidx_in'][i]
        m['dcmp'] = g['dcmp_in'][i]
        m['dinv'] = g['dinv_in'][i]
        in_maps.append(m)
    return in_maps


def _assemble_output(outs):
    h2 = np.empty((N, HID), np.float32)
    for i in range(C):
        h2[i * P:(i + 1) * P] = outs[i]['houtT'][:, :P].T
    return h2


def kernel(**inputs):
    edge_index = np.asarray(inputs['edge_index'])
    g, runner = _prepare(edge_index, nreps=1)
    in_maps = _make_in_maps(inputs, g)
    runner.upload(in_maps)
    outs = runner.call_outputs()
    return _assemble_output(outs)
